# revision 1
# baseline (speedup 1.0000x reference)
"""AdaAttNStar fused kernel for 8 TRN2 NeuronCores (v2).

Algebraic collapse: A = Q^T K is never built; M = (V K^T) Q and
S2 = (V^2 K^T) Q - M^2 reduce everything to channel Grams ([B,3,3]),
global norm stats, and an elementwise epilogue on each core's N-slice.

v2 engine plan (per trace analysis of the v1 kernel):
 - Inputs are cast to bf16 on the host (2e-2 tolerance; measured ~1e-3).
 - copy-A [128, (fs|fsp) c 144] feeds the 18 Gram STTs on DVE (2x mode).
 - copy-B re-layouts all 4 tensors as 96 partitions x 2304 (partition =
   (tensor,channel,batch)-group x 4) so ONE ACT Identity+accum and ONE
   ACT Square+accum produce all 24 sums/sumsqs - the two 1.9us DVE
   grouped reduces of v1 disappear.
 - per-b Gram separation via two masked-copy STTs; one ones-matrix
   matmul pushes all stats to ALL 128 partitions, so the scalar chain
   runs replicated and the v1 PE broadcast roundtrips are gone.
 - scalar chain fused (no eps, one reciprocal for ac+akq, broadcast APs
   instead of replicated tiles); epilogue split DVE/gpsimd; ACT sqrt
   overlapped with independent DVE ops.  Single ACT table (sqrt group).
 - no end-of-kernel dOUT wait / sem clear: the NEFF postamble resets all
   semaphores anyway (verified against re-execution in testing).
"""

import numpy as np
import ml_dtypes

import concourse.bass as bass
import concourse.bacc as bacc
from concourse import mybir
from concourse.bass_utils import run_bass_kernel_spmd

B, C, N = 2, 3, 9216
P, F = 128, 72            # gram layout: partition = b*64 + p, free = (c, 144)
F2 = 2 * F
NCORES = 8
NS, FS = N // NCORES, F // NCORES     # 1152, 9
MTOT = B * N              # 18432
GP, GK = 24, 4            # copy-B: 24 (t,c,b) groups x 4 partitions
assert GP * GK == 96
GFD = 9216 // GK          # 2304 free elems per copy-B partition
f32 = mybir.dt.float32
bf16 = mybir.dt.bfloat16
Alu = mybir.AluOpType
Act = mybir.ActivationFunctionType
X = mybir.AxisListType.X

FULL_INPUTS = ["F_c", "F_s", "F_c_previous", "F_s_previous"]
TORD = ["F_s", "F_c_previous", "F_s_previous", "F_c"]  # copy-B group order


def build():
    nc = bacc.Bacc("TRN2", target_bir_lowering=False, debug=False,
                   num_devices=NCORES)
    dr = {
        "inA": nc.dram_tensor("inA", [P, 2 * C * F2], bf16,
                              kind="ExternalInput"),
        "inB": nc.dram_tensor("inB", [P, GFD], mybir.dt.float8e4,
                              kind="ExternalInput"),
        "inS": nc.dram_tensor("inS", [P, 2 * B * C * FS + 48], f32,
                              kind="ExternalInput"),
    }
    out_sl = nc.dram_tensor("out_sl", [P, B, C, FS], f32,
                            kind="ExternalOutput")
    import os as _os
    dbg = _os.environ.get("KDEBUG") == "1"
    if dbg:
        dr_dbg = {
            "dbg_sc": nc.dram_tensor("dbg_sc", [P, 264], f32,
                                     kind="ExternalOutput"),
            "dbg_accG": nc.dram_tensor("dbg_accG", [P, 104], f32,
                                       kind="ExternalOutput"),
            "dbg_acc2": nc.dram_tensor("dbg_acc2", [P, 2], f32,
                                       kind="ExternalOutput"),
            "dbg_mt": nc.dram_tensor("dbg_mt", [P, 108], f32,
                                     kind="ExternalOutput"),
            "dbg_cnt": nc.dram_tensor("dbg_cnt", [P, 54], f32,
                                      kind="ExternalOutput"),
            "dbg_fcp3": nc.dram_tensor("dbg_fcp3", [P, 162], f32,
                                       kind="ExternalOutput"),
            "dbg_stt": nc.dram_tensor("dbg_stt", [P, 54], f32,
                                      kind="ExternalOutput"),
        }

    sb = lambda name, shape, dt=f32: nc.alloc_sbuf_tensor(name, shape, dt).ap()
    tA = sb("tA", [P, 2 * C * F2], bf16)      # fs | fsp, [128,(t c 144)]
    t_fs, t_fsp = tA[:, 0:C * F2], tA[:, C * F2:2 * C * F2]
    tB = sb("tB", [P, GFD], mybir.dt.float8e4)  # (t,c,b)-grouped
    bscr = sb("bscr", [P, GFD], bf16)         # ACT out scratch
    v2 = sb("v2", [P, C * F2], bf16)          # fs^2 in gram layout
    tS = sb("tS", [P, 2 * B * C * FS + 48])   # fcp_sl | fc_sl | maskpat
    fcp_sl = tS[:, 0:B * C * FS]
    fc_sl = tS[:, B * C * FS:2 * B * C * FS]
    maskpat = tS[:, 2 * B * C * FS:]
    fcp3 = sb("fcp3", [P, B * C * C * FS])
    ones_mat = sb("ones_mat", [P, P])
    b0c = sb("b0c", [P, 1])
    b1c = sb("b1c", [P, 1])
    acc2 = sb("acc2", [P, 2])                 # ACT accums: J1 | J2
    accG = sb("accG", [P, 104])               # 0:18 R | 18:36 Rb0 | 36:54 Rb1
                                              # | 54:102 J48 (2 pad)
    wscr = [sb(f"wscr{k}", [P, F2], bf16) for k in range(2)]
    sc = sb("sc", [P, 264])
    prodA = sb("prodA", [P, B * C * C * FS])
    prodB = sb("prodB", [P, B * C * C * FS])
    red = sb("red", [P, 2 * B * C * FS])      # w0 | w1
    mt = sb("mt", [P, 2 * B * C * FS])        # mt0 | mt1
    msq = sb("msq", [P, B * C * FS])
    s2 = sb("s2", [P, B * C * FS])
    s2c = sb("s2c", [P, B * C * FS])
    stt = sb("stt", [P, B * C * FS])
    cnt = sb("cnt", [P, B * C * FS])
    out_t = sb("out_t", [P, B * C * FS])
    palA = nc.alloc_psum_tensor("palA", [P, 512], f32).ap()
    palB = nc.alloc_psum_tensor("palB", [P, 24], f32).ap()

    import contextlib
    ctx = contextlib.ExitStack()
    names = ["dA", "dB", "dS", "dOUT", "sACT", "sDVE", "sGP", "sPE"]
    S = {n: ctx.enter_context(nc.semaphore(n)) for n in names}
    dA, dB, dS, dOUT, sACT, sDVE, sGP, sPE = (S[n] for n in names)

    # gram-layout channel views
    def ch(ap_, c):
        return ap_[:, c * F2:(c + 1) * F2]

    nd = [0]
    J0 = 54  # accG col of first J column

    with nc.Block() as block:

        @block.sync
        def _(sync):
            sync.dma_start(tA[:], dr["inA"].ap()).then_inc(dA, 16)
            sync.dma_start(tS[:], dr["inS"].ap()).then_inc(dS, 16)
            sync.wait_ge(sDVE, ND_FINAL)
            sync.dma_start(
                out_sl.ap(),
                out_t.rearrange("p (b c f) -> p b c f", b=B, c=C)
            ).then_inc(dOUT, 16)
            if dbg:
                for nm, ap_ in [("dbg_sc", sc), ("dbg_accG", accG),
                                ("dbg_acc2", acc2), ("dbg_mt", mt),
                                ("dbg_cnt", cnt), ("dbg_fcp3", fcp3),
                                ("dbg_stt", stt)]:
                    sync.dma_start(dr_dbg[nm].ap(), ap_).then_inc(dOUT, 16)

        @block.scalar
        def _(scalar):
            scalar.dma_start(tB[:], dr["inB"].ap()).then_inc(dB, 16)
            scalar.wait_ge(dA, 16)
            scalar.activation(v2[:], t_fs[:], Act.Square).then_inc(sACT)  # 1
            scalar.wait_ge(dB, 16)
            scalar.activation(bscr[:], tB[:], Act.Copy,
                              accum_out=acc2[:, 0:1]).then_inc(sACT)      # 2
            scalar.activation(bscr[:], tB[:], Act.Square,
                              accum_out=acc2[:, 1:2]).then_inc(sACT)      # 3
            scalar.wait_ge(sDVE, ND_VARM)
            scalar.activation(sc[:, 120:129], sc[:, 111:120], Act.Sqrt,
                              scale=1.0 / (MTOT - 1)).then_inc(sACT)      # 4
            scalar.wait_ge(sDVE, ND_S2C)
            scalar.activation(stt[:], s2c[:], Act.Sqrt).then_inc(sACT)    # 5

        @block.gpsimd
        def _(gp):
            gp.memset(ones_mat[:], 1.0).then_inc(sGP)   # 1
            gp.memset(b0c[0:64], 1.0).then_inc(sGP)     # 2
            gp.memset(b0c[64:128], 0.0).then_inc(sGP)   # 3
            gp.memset(b1c[0:64], 0.0).then_inc(sGP)     # 4
            gp.memset(b1c[64:128], 1.0).then_inc(sGP)   # 5
            gp.wait_ge(dS, 16)
            # fcp3[b,i,j,f] = fcp_sl[b,j,f]  (i broadcast), per b
            CF = C * FS
            for b in range(B):
                gp.tensor_copy(
                    fcp3[:, b * C * CF:(b + 1) * C * CF]
                    .rearrange("p (i jf) -> p i jf", i=C),
                    fcp_sl[:, b * CF:(b + 1) * CF]
                    .unsqueeze(1).broadcast_to((P, C, CF))
                ).then_inc(sGP)                          # 6, 7
            # prod1 = fcp3 * H[w=1] while DVE does w=0
            gp.wait_ge(sDVE, ND_H)
            gp.wait_ge(sGP, 7)
            gp.tensor_mul(
                prodB.rearrange("p (g f) -> p g f", f=FS),
                fcp3.rearrange("p (g f) -> p g f", f=FS),
                sc[:, 228:246].unsqueeze(2).broadcast_to((P, 18, FS))
            ).then_inc(sGP)                              # 8
            gp.wait_ge(sDVE, ND_CHAIN)
            for b in range(B):
                sl_ = slice(b * CF, (b + 1) * CF)
                gp.tensor_mul(
                    cnt[:, sl_].rearrange("p (c f) -> p c f", c=C),
                    fc_sl[:, sl_].rearrange("p (c f) -> p c f", c=C),
                    sc[:, 132:135].unsqueeze(2).broadcast_to((P, C, FS))
                ).then_inc(sGP)                          # 9, 10
            gp.wait_ge(sGP, 10)
            for b in range(B):
                sl_ = slice(b * CF, (b + 1) * CF)
                gp.tensor_sub(
                    cnt[:, sl_].rearrange("p (c f) -> p c f", c=C),
                    cnt[:, sl_].rearrange("p (c f) -> p c f", c=C),
                    sc[:, 258:261].unsqueeze(2).broadcast_to((P, C, FS))
                ).then_inc(sGP)                          # 11, 12

        @block.vector
        def _(V):
            def dv(inst):
                nd[0] += 1
                inst.then_inc(sDVE, 1)
                return nd[0]

            def wv():
                # relaxed ordering: same-engine RAW needs an explicit wait
                V.wait_ge(sDVE, nd[0])

            V.wait_ge(dA, 16)
            # R1: <fs_i, fsp_j> -> accG[:, 0:9]  (TTR: TT-class, 2x bf16)
            for i in range(C):
                for j in range(C):
                    q = i * C + j
                    dv(V.scalar_tensor_tensor(
                        out=wscr[q % 2][:], in0=ch(t_fs, i), scalar=1.0,
                        in1=ch(t_fsp, j), op0=Alu.mult, op1=Alu.mult,
                        accum_out=accG[:, q:q + 1]))
            V.wait_ge(sACT, 1)
            # R2: <v2_i, fsp_j> -> accG[:, 9:18]
            for i in range(C):
                for j in range(C):
                    q = 9 + i * C + j
                    dv(V.scalar_tensor_tensor(
                        out=wscr[q % 2][:], in0=ch(v2, i), scalar=1.0,
                        in1=ch(t_fsp, j), op0=Alu.mult, op1=Alu.mult,
                        accum_out=accG[:, q:q + 1]))
            assert nd[0] == 18
            # b-masked copies of R
            V.wait_ge(sGP, 5)
            wv()
            for k, bc_ in enumerate([b0c, b1c]):
                dv(V.scalar_tensor_tensor(
                    out=accG[:, 18 + 18 * k:36 + 18 * k],
                    in0=accG[:, 0:18], scalar=1.0,
                    in1=bc_.broadcast_to((P, 18)),
                    op0=Alu.mult, op1=Alu.mult))
            # scatter ACT accums into masked columns, J1 then J2
            V.wait_ge(sACT, 2)
            V.wait_ge(dS, 16)
            k = dv(V.tensor_mul(
                accG[:, J0:J0 + 24],
                acc2[:, 0:1].broadcast_to((P, 24)),
                maskpat[:, 0:24]))
            assert k == ND_R48A
            V.wait_ge(sACT, 3)
            k = dv(V.tensor_mul(
                accG[:, J0 + 24:J0 + 48],
                acc2[:, 1:2].broadcast_to((P, 24)),
                maskpat[:, 24:48]))
            assert k == ND_R48B
            # matmul -> stats on all partitions, then the scalar chain
            V.wait_ge(sPE, 1)
            dv(V.tensor_copy(sc[:, 0:60], palA[:, 0:60]))
            wv()
            # J col (within 24): g = t*6 + c*2 + b ; sc J1 base 36, J2 base 60
            up = sc[:, 84:93]    # sums  (fcp, fsp, fc) pooled over b
            ssp = sc[:, 93:102]  # sumsq (fcp, fsp, fc)
            j1 = sc[:, 36:60]
            j2 = sc[:, 60:84]
            dv(V.tensor_add(
                up.rearrange("p (t c) -> p t c", t=3),
                j1[:, 6:24].rearrange("p (t c b) -> p t c b", t=3,
                                      c=C)[:, :, :, 0],
                j1[:, 6:24].rearrange("p (t c b) -> p t c b", t=3,
                                      c=C)[:, :, :, 1]))
            wv()
            dv(V.tensor_mul(sc[:, 102:111], up, up))
            V.wait_ge(sPE, 2)
            dv(V.tensor_copy(sc[:, 60:84], palB[:, 0:24]))
            wv()
            dv(V.tensor_add(
                ssp.rearrange("p (t c) -> p t c", t=3),
                j2[:, 6:24].rearrange("p (t c b) -> p t c b", t=3,
                                      c=C)[:, :, :, 0],
                j2[:, 6:24].rearrange("p (t c b) -> p t c b", t=3,
                                      c=C)[:, :, :, 1]))
            wv()
            k = dv(V.scalar_tensor_tensor(
                out=sc[:, 111:120], in0=sc[:, 102:111],
                scalar=-1.0 / MTOT, in1=ssp, op0=Alu.mult, op1=Alu.add))
            assert k == ND_VARM
            # GRID/RC per (b,w) while ACT computes std (3D AP limit)
            for b in range(2):
                for w in range(2):
                    dv(V.tensor_mul(
                        sc[:, 138 + 18 * w + 9 * b:147 + 18 * w + 9 * b]
                        .rearrange("p (i j) -> p i j", i=C),
                        sc[:, 36 + 24 * w + b:42 + 24 * w + b]
                        .rearrange("p (i b2) -> p i b2", i=C)[:, :, 0:1]
                        .broadcast_to((P, C, C)),
                        sc[:, 87:90].unsqueeze(1).broadcast_to((P, C, C))))
            wv()
            for b in range(2):
                for w in range(2):
                    dv(V.scalar_tensor_tensor(
                        out=sc[:, 174 + 18 * w + 9 * b:183 + 18 * w + 9 * b]
                        .rearrange("p (i j) -> p i j", i=C),
                        in0=sc[:, 138 + 18 * w + 9 * b:
                               147 + 18 * w + 9 * b]
                        .rearrange("p (i j) -> p i j", i=C),
                        scalar=-1.0 / MTOT,
                        in1=sc[:, 18 * b + 9 * w:18 * b + 9 * w + 9]
                        .rearrange("p (i j) -> p i j", i=C),
                        op0=Alu.mult, op1=Alu.add))
            V.wait_ge(sACT, 4)
            # skq = std_fcp*std_fsp ; [ac|akq] = 1/[std_fc|skq]
            dv(V.tensor_mul(sc[:, 129:132], sc[:, 120:123],
                            sc[:, 123:126]))
            wv()
            dv(V.reciprocal(sc[:, 132:138], sc[:, 126:132]))
            wv()
            k = dv(V.tensor_mul(
                sc[:, 210:246].rearrange("p (g j) -> p g j", j=C),
                sc[:, 174:210].rearrange("p (g j) -> p g j", j=C),
                sc[:, 135:138].unsqueeze(1).broadcast_to((P, 12, C))))
            assert k == ND_H
            wv()
            dv(V.scalar_tensor_tensor(
                out=sc[:, 138:174].rearrange("p (g j) -> p g j", j=C),
                in0=sc[:, 210:246].rearrange("p (g j) -> p g j", j=C),
                scalar=1.0 / MTOT,
                in1=sc[:, 84:87].unsqueeze(1).broadcast_to((P, 12, C)),
                op0=Alu.mult, op1=Alu.mult))
            wv()
            dv(V.reduce_sum(
                sc[:, 246:258],
                sc[:, 138:174].rearrange("p (g j) -> p g j", j=C), axis=X))
            k = dv(V.scalar_tensor_tensor(
                out=sc[:, 258:261], in0=sc[:, 90:93], scalar=1.0 / MTOT,
                in1=sc[:, 132:135], op0=Alu.mult, op1=Alu.mult))
            assert k == ND_CHAIN
            # epilogue
            V.wait_ge(sGP, 7)
            dv(V.tensor_mul(
                prodA.rearrange("p (g f) -> p g f", f=FS),
                fcp3.rearrange("p (g f) -> p g f", f=FS),
                sc[:, 210:228].unsqueeze(2).broadcast_to((P, 18, FS))))
            wv()
            dv(V.reduce_sum(
                red[:, 0:B * C * FS].rearrange("p (g f) -> p g f", f=FS),
                prodA.rearrange("p (g j f) -> p g f j", j=C, f=FS), axis=X))
            V.wait_ge(sGP, 8)
            dv(V.reduce_sum(
                red[:, B * C * FS:].rearrange("p (g f) -> p g f", f=FS),
                prodB.rearrange("p (g j f) -> p g f j", j=C, f=FS), axis=X))
            wv()
            dv(V.scalar_tensor_tensor(
                out=mt.rearrange("p (g f) -> p g f", f=FS),
                in0=sc[:, 246:258].unsqueeze(2).broadcast_to((P, 12, FS)),
                scalar=-1.0,
                in1=red.rearrange("p (g f) -> p g f", f=FS),
                op0=Alu.mult, op1=Alu.add))
            wv()
            dv(V.tensor_mul(msq[:], mt[:, 0:B * C * FS],
                            mt[:, 0:B * C * FS]))
            wv()
            dv(V.scalar_tensor_tensor(
                out=s2[:], in0=msq[:], scalar=-1.0,
                in1=mt[:, B * C * FS:], op0=Alu.mult, op1=Alu.add))
            wv()
            k = dv(V.tensor_scalar_max(s2c[:], s2[:], 0.0))
            assert k == ND_S2C
            V.wait_ge(sACT, 5)
            V.wait_ge(sGP, 12)
            dv(V.tensor_mul(out_t[:], stt[:], cnt[:]))
            wv()
            k = dv(V.tensor_add(out_t[:], out_t[:], mt[:, 0:B * C * FS]))
            assert k == ND_FINAL

        @block.tensor
        def _(te):
            te.wait_ge(sGP, 1)
            te.wait_ge(sDVE, ND_R48A)
            te.matmul(palA[:, 0:60], ones_mat[:], accG[:, 18:78],
                      start=True, stop=True).then_inc(sPE)
            te.wait_ge(sDVE, ND_R48B)
            te.matmul(palB[:, 0:24], ones_mat[:], accG[:, 78:102],
                      start=True, stop=True).then_inc(sPE)

    # out-DMA must be complete before the NEFF ends (runtime tears down
    # DMA rings); overlap the wait with the exit barrier on gpsimd.
    nc.gpsimd.wait_ge(dOUT, 16 * (8 if dbg else 1))

    ctx.pop_all()
    nc.compile()
    return nc


# sDVE milestones (emission order above)
ND_R48A = 21
ND_R48B = 22
ND_VARM = ND_R48B + 6   # copy1, UP, q, copy2, SSP, varm
ND_H = ND_VARM + 11     # GRIDx4, RCx4, skq, recip, H
ND_CHAIN = ND_H + 3     # HM, H0, mcac
ND_S2C = ND_CHAIN + 7   # prodA, red0, red1, mt, msq, s2, s2c
ND_FINAL = ND_S2C + 2   # om, out


_NC = None


def _get_nc():
    global _NC
    if _NC is None:
        _NC = build()
    return _NC


def _pmajor(x, f):
    # [B, C, n] -> [128, B, C, f] with n = p*f + j
    return np.ascontiguousarray(
        x.reshape(B, C, P, f).transpose(2, 0, 1, 3))


def _pmajor_b(x):
    # [B, C, n] -> [128, C, 144]: partition = b*64 + p, n = p*144 + f
    return np.ascontiguousarray(
        x.reshape(B, C, 64, F2).transpose(0, 2, 1, 3).reshape(P, C, F2))


def make_in_maps(inputs):
    full = {k: np.asarray(inputs[k], dtype=np.float32).reshape(B, C, N)
            for k in FULL_INPUTS}
    # copy-A: fs | fsp gram layout, bf16
    inA = np.concatenate(
        [_pmajor_b(full["F_s"]).reshape(P, C * F2),
         _pmajor_b(full["F_s_previous"]).reshape(P, C * F2)],
        axis=1).astype(ml_dtypes.bfloat16)
    # copy-B: 96 partitions = (t,c,b) group * 4, free = 2304
    inB = np.zeros((P, GFD), np.float32)
    for t, name in enumerate(TORD):
        for c in range(C):
            for b in range(B):
                g = t * 6 + c * 2 + b
                inB[g * GK:(g + 1) * GK] = full[name][b, c].reshape(GK, GFD)
    inB = inB.astype(ml_dtypes.float8_e4m3fn)
    # maskpat [128, 48]: col = j*24+g, 1.0 iff p//4 == g (p<96)
    mp = np.zeros((P, 48), np.float32)
    for g in range(GP):
        mp[g * GK:(g + 1) * GK, g] = 1.0
        mp[g * GK:(g + 1) * GK, 24 + g] = 1.0
    in_maps = []
    for r in range(NCORES):
        sl = slice(r * NS, (r + 1) * NS)
        inS = np.concatenate(
            [_pmajor(full["F_c_previous"][:, :, sl], FS).reshape(P, -1),
             _pmajor(full["F_c"][:, :, sl], FS).reshape(P, -1),
             mp], axis=1).astype(np.float32)
        in_maps.append({"inA": inA, "inB": inB, "inS": inS})
    return in_maps


def kernel(**inputs):
    nc = _get_nc()
    res = run_bass_kernel_spmd(nc, make_in_maps(inputs),
                               core_ids=list(range(NCORES)))
    return np.concatenate(
        [res.results[r]["out_sl"].transpose(1, 2, 0, 3).reshape(B, C, NS)
         for r in range(NCORES)], axis=2)

